# revision 37
# baseline (speedup 1.0000x reference)
"""MaxSim (ColBERT late-interaction) retrieval scoring on Trainium2.

scores[q, d] = sum_m max_n <Q[q,m,:], D[d,n,:]>
Q [32, 32, 128], D [256, 180, 128] -> scores [32, 256] fp32.

Sharding: doc axis split across 8 NeuronCores (32 docs each), full Q
replicated; per-core partial scores [32q, 32d] concatenated on host.

Per-core design:
  - 128 SBUF partitions = 4 queries x 32 query-tokens (8 query "groups").
  - PE: per group, Q-group [128h, 128qm] stationary (redundant LDWEIGHTS
    deduped post-Tile); doc tokens stream as the moving operand in fp16 ->
    PSUM sim tiles [128, 1440] (8 docs x 180 toks).
  - Max over doc tokens, split across two engines to beat the DVE-only
    1 elem/cycle/lane tensor_reduce bottleneck (both engines drain PSUM
    concurrently, block-interleaved within each group):
      block 0 of each group: DVE reduce_max directly from PSUM;
      blocks 1-3: ScalarE copy-converts PSUM fp32 -> SBUF fp16, then DVE
        folds pairwise with tensor_tensor max in 2x_1p mode (2 elem/
        cycle/lane, overlapping even-offset windows 180->92->46->24->12)
        and one small strided reduce_max.
  - PE: per-group selector matmul sums each query's 32 token-maxes:
    sel[128,4].T @ maxvals[128, 32d] -> outps[4, 32] -> DMA as [32q, 32d].
"""

import numpy as np
from contextlib import ExitStack

import concourse.mybir as mybir
import concourse.tile as tile
from concourse import bacc
from concourse.bass_utils import run_bass_kernel_spmd

H = 128            # head dim (contraction)
NQ, M = 32, 32     # queries, query tokens
NDOC, NTOK = 256, 180
NCORES = 8
DSHARD = NDOC // NCORES      # 32 docs per core
GROUPS, QPG = 8, 4           # query groups of 4; 4*32 = 128 partitions
BLK_DOCS = 8                 # docs per PSUM block
NBLK = DSHARD // BLK_DOCS    # 4 blocks per group
BLK = BLK_DOCS * NTOK        # 1440 sim columns per block
GCOLS = DSHARD * NTOK        # 5760 doc-token columns per core
FOLD_DOCS = DSHARD - BLK_DOCS  # 24 docs per group take the fold route

# overlapping pairwise-fold splits: (len, off0, off1); all offsets even so
# fp16 packed pairs stay 4B-aligned for the DVE 2x_1p mode.
FOLD_PLAN = [(92, 0, 88), (46, 0, 46), (24, 0, 22), (12, 0, 12)]

_CACHE = {}


# combined input tensor layout: [qt (1024) | dt (5760) | sel (4)] = 6788 cols
QT_OFF = 0
DT_OFF = GROUPS * 128          # 1024
SEL_OFF = DT_OFF + GCOLS       # 6784
IN_COLS = SEL_OFF + QPG        # 6788
# DMA plan: qt on the sync queue; doc blocks on the scalar queue in
# consumption order (block 0 first, so it lands in parallel with qt).
# Each slice <= 2560 cols so every descriptor stays one per partition
# row under the 5120B cap. No gpsimd-issued DMA: first SWDGE use pays a
# ~6us Q7 code load and a dge_drain at exit.
DMA_PLAN = [("sync", 0, DT_OFF),
            ("scalar", DT_OFF, DT_OFF + BLK),
            ("scalar", DT_OFF + BLK, DT_OFF + 3 * BLK),
            ("scalar", DT_OFF + 3 * BLK, IN_COLS)]


def _ldw_key(inst):
    w = inst.ins[0]
    return (str(w.memref), w.offset, str(w.ap), str(w.dtype))


def _dedupe_ldweights(nc):
    """Drop LDWEIGHTS whose stationary operand is already loaded.

    Tile lowers every matmul to an Ldweights+Matmult pair; our inner loops
    issue 12 matmuls per Q-group against the same stationary operand, and
    the redundant reloads serialize the PE (~480ns/matmul vs ~215ns).
    Walrus codegen emits exactly the BIR instructions, so removing a
    duplicate here removes the HW reload. Sems on a removed Ldweights are
    merged onto the following instruction (same engine, later point - the
    waits still guard the matmul, the updates still fire)."""
    for blk in nc.m.functions[0].blocks:
        insts = blk.instructions
        last_key = None
        keep = []
        pending_waits, pending_updates = [], []
        for inst in insts:
            if inst.opcode == "Ldweights":
                key = _ldw_key(inst)
                if key == last_key:
                    si = inst.sync_info
                    if si is not None:
                        pending_waits.extend(si.on_wait or [])
                        pending_updates.extend(si.on_update or [])
                    continue        # drop duplicate
                last_key = key
            elif inst.opcode == "Matmult":
                if pending_waits or pending_updates:
                    si = inst.sync_info
                    if si is None:
                        import concourse.mybir as _mybir
                        inst.sync_info = _mybir.SyncInfo(
                            on_wait=pending_waits, on_update=pending_updates)
                    else:
                        si.on_wait = list(si.on_wait or []) + pending_waits
                        si.on_update = list(si.on_update or []) + pending_updates
                    pending_waits, pending_updates = [], []
            elif getattr(inst, "engine", None) == mybir.EngineType.PE:
                last_key = None
            keep.append(inst)
        assert not pending_waits and not pending_updates
        if len(keep) != len(insts):
            blk.instructions[:] = keep


def _build():
    nc = bacc.Bacc(None, target_bir_lowering=False)
    f16 = mybir.dt.float16
    inp = nc.dram_tensor("inp", [H, IN_COLS], f16, kind="ExternalInput")
    scores = nc.dram_tensor("scores", [NQ, DSHARD], mybir.dt.float32,
                            kind="ExternalOutput")

    with ExitStack() as ctx:
        tc = ctx.enter_context(tile.TileContext(nc))
        singles = ctx.enter_context(tc.tile_pool(name="singles", bufs=1))
        psums = ctx.enter_context(tc.tile_pool(name="psums", bufs=2,
                                               space="PSUM"))
        gbufs = ctx.enter_context(tc.tile_pool(name="gbufs", bufs=3))
        folds = ctx.enter_context(tc.tile_pool(name="folds", bufs=2))
        outp = ctx.enter_context(tc.tile_pool(name="outp", bufs=1,
                                              space="PSUM"))

        in_sb = singles.tile([H, IN_COLS], f16)
        q_sb = in_sb[:, QT_OFF:QT_OFF + GROUPS * 128]
        d_sb = in_sb[:, DT_OFF:DT_OFF + GCOLS]
        sel_sb = in_sb[:, SEL_OFF:SEL_OFF + QPG]
        maxv = singles.tile([128, GROUPS, DSHARD], f16)

        for eng_name, lo, hi in DMA_PLAN:
            eng = {"sync": nc.sync, "scalar": nc.scalar,
                   "gpsimd": nc.gpsimd}[eng_name]
            eng.dma_start(out=in_sb[:, lo:hi], in_=inp[:, lo:hi])

        mm_splits = [(0, 512), (512, 512), (1024, BLK - 1024)]

        def sim_block(g, b):
            """Matmul one [128, 1440] sim block for group g into PSUM."""
            qg = q_sb[:, g * 128:(g + 1) * 128]
            ps = psums.tile([128, BLK], mybir.dt.float32, tag="ps",
                            name=f"ps_{g}_{b}")
            for (o, w) in mm_splits:
                nc.tensor.matmul(ps[:, o:o + w], qg,
                                 d_sb[:, b * BLK + o: b * BLK + o + w],
                                 start=True, stop=True)
            return ps

        # Per group: block 0 -> DVE reduce_max straight from PSUM (emitted
        # first so DVE starts as early as possible); blocks 1-3 -> ScalarE
        # copy-convert to fp16 SBUF, then a DVE fold tree. The fold tree of
        # group g-1 is emitted AFTER group g's direct reduce, so on the DVE
        # queue each group's PSUM-freeing reduce runs before the (slot-free)
        # fold work - both engines drain PSUM concurrently.
        outps = outp.tile([QPG, GROUPS * DSHARD], mybir.dt.float32)

        def emit_selector(g):
            # sum each query's 32 token-maxes for this group's docs; the
            # tiny 4-col selector reload costs the PE only a few ns.
            nc.tensor.matmul(outps[:, g * DSHARD:(g + 1) * DSHARD], sel_sb,
                             maxv[:, g, :], start=True, stop=True)

        def emit_folds(g, buf, docs, doc_lo):
            src = buf
            for li, (ln, o0, o1) in enumerate(FOLD_PLAN):
                dst = folds.tile([128, docs, ln], f16,
                                 tag=f"fold{li}", name=f"fold{li}_{g}_{doc_lo}")
                nc.vector.tensor_tensor(
                    dst, src[:, :, o0:o0 + ln], src[:, :, o1:o1 + ln],
                    op=mybir.AluOpType.max)
                src = dst
            nc.vector.reduce_max(maxv[:, g, doc_lo:doc_lo + docs], src,
                                 axis=mybir.AxisListType.X)
            if doc_lo + docs == DSHARD:
                emit_selector(g)

        # Block 0 of each group -> DVE direct reduce (emitted before the
        # previous group's folds, so the PSUM-freeing reduce runs first on
        # the DVE queue). The last group folds per-block to shorten the
        # serial fold tail.
        def emit_direct(g, b):
            ps = sim_block(g, b)
            nc.vector.reduce_max(
                maxv[:, g, 0:BLK_DOCS],
                ps.rearrange("p (d n) -> p d n", n=NTOK),
                axis=mybir.AxisListType.X)

        prev = None
        for g in range(GROUPS):
            last = g == GROUPS - 1
            buf = gbufs.tile([128, FOLD_DOCS, NTOK], f16, tag="buf",
                             name=f"buf_{g}")
            for b in range(NBLK):
                if b == 0:
                    if g == 0:
                        emit_direct(0, 0)
                    elif g == 1:
                        pass       # hoisted into group 0's sequence below
                    else:
                        emit_direct(g, b)
                    if prev is not None:
                        emit_folds(g - 1, prev, FOLD_DOCS, BLK_DOCS)
                        prev = None
                else:
                    ps = sim_block(g, b)
                    nc.scalar.copy(
                        out=buf[:, (b - 1) * BLK_DOCS:b * BLK_DOCS, :],
                        in_=ps.rearrange("p (d n) -> p d n", n=NTOK))
                    if last:
                        emit_folds(g, buf[:, (b - 1) * BLK_DOCS:b * BLK_DOCS, :],
                                   BLK_DOCS, b * BLK_DOCS)
                    if g == 0 and b == 1:
                        # hoist group 1's direct block here: ScalarE gets
                        # its first copy one slot sooner and the DVE has a
                        # second direct reduce queued during pipeline fill
                        emit_direct(1, 0)
            if not last:
                prev = buf


        scores_sb = singles.tile([QPG, GROUPS * DSHARD], mybir.dt.float32)
        nc.scalar.copy(out=scores_sb, in_=outps)
        nc.sync.dma_start(
            out=scores[:, :].rearrange("(g j) d -> j g d", j=QPG),
            in_=scores_sb.rearrange("j (g d) -> j g d", g=GROUPS))
    _dedupe_ldweights(nc)
    nc.finalize()
    return nc


def _get_program():
    if "nc" not in _CACHE:
        _CACHE["nc"] = _build()
    return _CACHE["nc"]


def _prep_inputs(Q, D, q_mask, d_mask):
    Qm = np.asarray(Q, np.float32) * np.asarray(q_mask, np.float32)[..., None]
    Dm = np.asarray(D, np.float32) * np.asarray(d_mask, np.float32)[..., None]

    qt = Qm.reshape(GROUPS, QPG, M, H).transpose(3, 0, 1, 2).reshape(
        H, GROUPS * 128).astype(np.float16)
    sel = np.repeat(np.eye(QPG, dtype=np.float32), M, axis=0).astype(np.float16)

    in_maps = []
    for c in range(NCORES):
        Dc = Dm[c * DSHARD:(c + 1) * DSHARD]          # [32, 180, 128]
        dtc = Dc.transpose(2, 0, 1).reshape(H, GCOLS).astype(np.float16)
        inp = np.empty((H, IN_COLS), dtype=np.float16)
        inp[:, QT_OFF:QT_OFF + GROUPS * 128] = qt
        inp[:, DT_OFF:DT_OFF + GCOLS] = dtc
        inp[:, SEL_OFF:SEL_OFF + QPG] = sel
        in_maps.append({"inp": inp})
    return in_maps


def run(Q, D, q_mask, d_mask, trace=False, **spmd_kwargs):
    """Run the sharded kernel; returns (scores [32,256] fp32, BassKernelResults)."""
    nc = _get_program()
    in_maps = _prep_inputs(Q, D, q_mask, d_mask)
    res = run_bass_kernel_spmd(nc, in_maps, core_ids=list(range(NCORES)),
                               trace=trace, **spmd_kwargs)
    full = np.empty((NQ, NDOC), dtype=np.float32)
    for c in range(NCORES):
        full[:, c * DSHARD:(c + 1) * DSHARD] = res.results[c]["scores"]
    return full, res


def kernel(Q, D, q_mask, d_mask):
    out, _ = run(Q, D, q_mask, d_mask, trace=False)
    return out



# revision 38
# speedup vs baseline: 1.0504x; 1.0504x over previous
"""MaxSim (ColBERT late-interaction) retrieval scoring on Trainium2.

scores[q, d] = sum_m max_n <Q[q,m,:], D[d,n,:]>
Q [32, 32, 128], D [256, 180, 128] -> scores [32, 256] fp32.

Sharding: doc axis split across 8 NeuronCores (32 docs each), full Q
replicated; per-core partial scores [32q, 32d] concatenated on host.

Per-core design:
  - 128 SBUF partitions = 4 queries x 32 query-tokens (8 query "groups").
  - PE: per group, Q-group [128h, 128qm] stationary (redundant LDWEIGHTS
    deduped post-Tile); doc tokens stream as the moving operand in fp16 ->
    PSUM sim tiles [128, 1440] (8 docs x 180 toks).
  - Max over doc tokens, split across two engines to beat the DVE-only
    1 elem/cycle/lane tensor_reduce bottleneck (both engines drain PSUM
    concurrently, block-interleaved within each group):
      block 0 of each group: DVE reduce_max directly from PSUM;
      blocks 1-3: ScalarE copy-converts PSUM fp32 -> SBUF fp16, then DVE
        folds pairwise with tensor_tensor max in 2x_1p mode (2 elem/
        cycle/lane, overlapping even-offset windows 180->92->46->24->12)
        and one small strided reduce_max.
  - PE: per-group selector matmul sums each query's 32 token-maxes:
    sel[128,4].T @ maxvals[128, 32d] -> outps[4, 32] -> DMA as [32q, 32d].
"""

import numpy as np
from contextlib import ExitStack

import concourse.mybir as mybir
import concourse.tile as tile
from concourse import bacc
from concourse.bass_utils import run_bass_kernel_spmd

H = 128            # head dim (contraction)
NQ, M = 32, 32     # queries, query tokens
NDOC, NTOK = 256, 180
NCORES = 8
DSHARD = NDOC // NCORES      # 32 docs per core
GROUPS, QPG = 8, 4           # query groups of 4; 4*32 = 128 partitions
BLK_DOCS = 8                 # docs per PSUM block
NBLK = DSHARD // BLK_DOCS    # 4 blocks per group
BLK = BLK_DOCS * NTOK        # 1440 sim columns per block
GCOLS = DSHARD * NTOK        # 5760 doc-token columns per core
FOLD_DOCS = DSHARD - BLK_DOCS  # 24 docs per group take the fold route

# overlapping pairwise-fold splits: (len, off0, off1); all offsets even so
# fp16 packed pairs stay 4B-aligned for the DVE 2x_1p mode.
FOLD_PLAN = [(92, 0, 88), (46, 0, 46), (24, 0, 22), (12, 0, 12)]

_CACHE = {}


# combined input tensor layout: [qt (1024) | dt (5760) | sel (4)] = 6788 cols
QT_OFF = 0
DT_OFF = GROUPS * 128          # 1024
SEL_OFF = DT_OFF + GCOLS       # 6784
IN_COLS = SEL_OFF + QPG        # 6788
# DMA plan: qt on the sync queue; doc blocks on the scalar queue in
# consumption order (block 0 first, so it lands in parallel with qt).
# Each slice <= 2560 cols so every descriptor stays one per partition
# row under the 5120B cap. No gpsimd-issued DMA: first SWDGE use pays a
# ~6us Q7 code load and a dge_drain at exit.
DMA_PLAN = [("sync", 0, DT_OFF),
            ("scalar", DT_OFF, DT_OFF + BLK),
            ("scalar", DT_OFF + BLK, DT_OFF + 3 * BLK),
            ("scalar", DT_OFF + 3 * BLK, IN_COLS)]


def _ldw_key(inst):
    w = inst.ins[0]
    return (str(w.memref), w.offset, str(w.ap), str(w.dtype))


def _dedupe_ldweights(nc):
    """Drop LDWEIGHTS whose stationary operand is already loaded.

    Tile lowers every matmul to an Ldweights+Matmult pair; our inner loops
    issue 12 matmuls per Q-group against the same stationary operand, and
    the redundant reloads serialize the PE (~480ns/matmul vs ~215ns).
    Walrus codegen emits exactly the BIR instructions, so removing a
    duplicate here removes the HW reload. Sems on a removed Ldweights are
    merged onto the following instruction (same engine, later point - the
    waits still guard the matmul, the updates still fire)."""
    for blk in nc.m.functions[0].blocks:
        insts = blk.instructions
        last_key = None
        keep = []
        pending_waits, pending_updates = [], []
        for inst in insts:
            if inst.opcode == "Ldweights":
                key = _ldw_key(inst)
                if key == last_key:
                    si = inst.sync_info
                    if si is not None:
                        pending_waits.extend(si.on_wait or [])
                        pending_updates.extend(si.on_update or [])
                    continue        # drop duplicate
                last_key = key
            elif inst.opcode == "Matmult":
                if pending_waits or pending_updates:
                    si = inst.sync_info
                    if si is None:
                        import concourse.mybir as _mybir
                        inst.sync_info = _mybir.SyncInfo(
                            on_wait=pending_waits, on_update=pending_updates)
                    else:
                        si.on_wait = list(si.on_wait or []) + pending_waits
                        si.on_update = list(si.on_update or []) + pending_updates
                    pending_waits, pending_updates = [], []
            elif getattr(inst, "engine", None) == mybir.EngineType.PE:
                last_key = None
            keep.append(inst)
        assert not pending_waits and not pending_updates
        if len(keep) != len(insts):
            blk.instructions[:] = keep


def _build():
    nc = bacc.Bacc(None, target_bir_lowering=False)
    f16 = mybir.dt.float16
    inp = nc.dram_tensor("inp", [H, IN_COLS], f16, kind="ExternalInput")
    scores = nc.dram_tensor("scores", [NQ, DSHARD], mybir.dt.float32,
                            kind="ExternalOutput")

    with ExitStack() as ctx:
        tc = ctx.enter_context(tile.TileContext(nc))
        singles = ctx.enter_context(tc.tile_pool(name="singles", bufs=1))
        psums = ctx.enter_context(tc.tile_pool(name="psums", bufs=2,
                                               space="PSUM"))
        gbufs = ctx.enter_context(tc.tile_pool(name="gbufs", bufs=3))
        folds = ctx.enter_context(tc.tile_pool(name="folds", bufs=2))
        outp = ctx.enter_context(tc.tile_pool(name="outp", bufs=1,
                                              space="PSUM"))

        in_sb = singles.tile([H, IN_COLS], f16)
        q_sb = in_sb[:, QT_OFF:QT_OFF + GROUPS * 128]
        d_sb = in_sb[:, DT_OFF:DT_OFF + GCOLS]
        sel_sb = in_sb[:, SEL_OFF:SEL_OFF + QPG]
        maxv = singles.tile([128, GROUPS, DSHARD], f16)

        for eng_name, lo, hi in DMA_PLAN:
            eng = {"sync": nc.sync, "scalar": nc.scalar,
                   "gpsimd": nc.gpsimd}[eng_name]
            eng.dma_start(out=in_sb[:, lo:hi], in_=inp[:, lo:hi])

        mm_splits = [(0, 512), (512, 512), (1024, BLK - 1024)]

        def sim_block(g, b):
            """Matmul one [128, 1440] sim block for group g into PSUM."""
            qg = q_sb[:, g * 128:(g + 1) * 128]
            ps = psums.tile([128, BLK], mybir.dt.float32, tag="ps",
                            name=f"ps_{g}_{b}")
            for (o, w) in mm_splits:
                nc.tensor.matmul(ps[:, o:o + w], qg,
                                 d_sb[:, b * BLK + o: b * BLK + o + w],
                                 start=True, stop=True)
            return ps

        # Per group: block 0 -> DVE reduce_max straight from PSUM (emitted
        # first so DVE starts as early as possible); blocks 1-3 -> ScalarE
        # copy-convert to fp16 SBUF, then a DVE fold tree. The fold tree of
        # group g-1 is emitted AFTER group g's direct reduce, so on the DVE
        # queue each group's PSUM-freeing reduce runs before the (slot-free)
        # fold work - both engines drain PSUM concurrently.
        outps = outp.tile([QPG, GROUPS * DSHARD], mybir.dt.float32)

        def emit_selector(g):
            # sum each query's 32 token-maxes for this group's docs; the
            # tiny 4-col selector reload costs the PE only a few ns.
            nc.tensor.matmul(outps[:, g * DSHARD:(g + 1) * DSHARD], sel_sb,
                             maxv[:, g, :], start=True, stop=True)

        def emit_folds(g, buf, docs, doc_lo):
            src = buf
            for li, (ln, o0, o1) in enumerate(FOLD_PLAN):
                dst = folds.tile([128, docs, ln], f16,
                                 tag=f"fold{li}", name=f"fold{li}_{g}_{doc_lo}")
                nc.vector.tensor_tensor(
                    dst, src[:, :, o0:o0 + ln], src[:, :, o1:o1 + ln],
                    op=mybir.AluOpType.max)
                src = dst
            nc.vector.reduce_max(maxv[:, g, doc_lo:doc_lo + docs], src,
                                 axis=mybir.AxisListType.X)
            if doc_lo + docs == DSHARD:
                emit_selector(g)

        # Block 0 of each group -> DVE direct reduce (emitted before the
        # previous group's folds, so the PSUM-freeing reduce runs first on
        # the DVE queue). The last group folds per-block to shorten the
        # serial fold tail.
        def emit_direct(g, b):
            ps = sim_block(g, b)
            nc.vector.reduce_max(
                maxv[:, g, 0:BLK_DOCS],
                ps.rearrange("p (d n) -> p d n", n=NTOK),
                axis=mybir.AxisListType.X)

        prev = None
        for g in range(GROUPS):
            last = g == GROUPS - 1
            buf = gbufs.tile([128, FOLD_DOCS, NTOK], f16, tag="buf",
                             name=f"buf_{g}")
            for b in range(NBLK):
                if b == 0:
                    if g == 0:
                        # hoist the first two groups' direct blocks so the
                        # DVE has queued work while the pipeline fills
                        emit_direct(0, 0)
                        emit_direct(1, 0)
                    elif g == 1:
                        pass       # already emitted
                    else:
                        emit_direct(g, b)
                    if prev is not None:
                        emit_folds(g - 1, prev, FOLD_DOCS, BLK_DOCS)
                        prev = None
                else:
                    ps = sim_block(g, b)
                    nc.scalar.copy(
                        out=buf[:, (b - 1) * BLK_DOCS:b * BLK_DOCS, :],
                        in_=ps.rearrange("p (d n) -> p d n", n=NTOK))
                    if last:
                        emit_folds(g, buf[:, (b - 1) * BLK_DOCS:b * BLK_DOCS, :],
                                   BLK_DOCS, b * BLK_DOCS)
            if not last:
                prev = buf


        scores_sb = singles.tile([QPG, GROUPS * DSHARD], mybir.dt.float32)
        nc.scalar.copy(out=scores_sb, in_=outps)
        nc.sync.dma_start(
            out=scores[:, :].rearrange("(g j) d -> j g d", j=QPG),
            in_=scores_sb.rearrange("j (g d) -> j g d", g=GROUPS))
    _dedupe_ldweights(nc)
    nc.finalize()
    return nc


def _get_program():
    if "nc" not in _CACHE:
        _CACHE["nc"] = _build()
    return _CACHE["nc"]


def _prep_inputs(Q, D, q_mask, d_mask):
    Qm = np.asarray(Q, np.float32) * np.asarray(q_mask, np.float32)[..., None]
    Dm = np.asarray(D, np.float32) * np.asarray(d_mask, np.float32)[..., None]

    qt = Qm.reshape(GROUPS, QPG, M, H).transpose(3, 0, 1, 2).reshape(
        H, GROUPS * 128).astype(np.float16)
    sel = np.repeat(np.eye(QPG, dtype=np.float32), M, axis=0).astype(np.float16)

    in_maps = []
    for c in range(NCORES):
        Dc = Dm[c * DSHARD:(c + 1) * DSHARD]          # [32, 180, 128]
        dtc = Dc.transpose(2, 0, 1).reshape(H, GCOLS).astype(np.float16)
        inp = np.empty((H, IN_COLS), dtype=np.float16)
        inp[:, QT_OFF:QT_OFF + GROUPS * 128] = qt
        inp[:, DT_OFF:DT_OFF + GCOLS] = dtc
        inp[:, SEL_OFF:SEL_OFF + QPG] = sel
        in_maps.append({"inp": inp})
    return in_maps


def run(Q, D, q_mask, d_mask, trace=False, **spmd_kwargs):
    """Run the sharded kernel; returns (scores [32,256] fp32, BassKernelResults)."""
    nc = _get_program()
    in_maps = _prep_inputs(Q, D, q_mask, d_mask)
    res = run_bass_kernel_spmd(nc, in_maps, core_ids=list(range(NCORES)),
                               trace=trace, **spmd_kwargs)
    full = np.empty((NQ, NDOC), dtype=np.float32)
    for c in range(NCORES):
        full[:, c * DSHARD:(c + 1) * DSHARD] = res.results[c]["scores"]
    return full, res


def kernel(Q, D, q_mask, d_mask):
    out, _ = run(Q, D, q_mask, d_mask, trace=False)
    return out



# revision 39
# speedup vs baseline: 1.0574x; 1.0066x over previous
"""MaxSim (ColBERT late-interaction) retrieval scoring on Trainium2.

scores[q, d] = sum_m max_n <Q[q,m,:], D[d,n,:]>
Q [32, 32, 128], D [256, 180, 128] -> scores [32, 256] fp32.

Sharding: doc axis split across 8 NeuronCores (32 docs each), full Q
replicated; per-core partial scores [32q, 32d] concatenated on host.

Per-core design:
  - 128 SBUF partitions = 4 queries x 32 query-tokens (8 query "groups").
  - PE: per group, Q-group [128h, 128qm] stationary (redundant LDWEIGHTS
    deduped post-Tile); doc tokens stream as the moving operand in fp16 ->
    PSUM sim tiles [128, 1440] (8 docs x 180 toks).
  - Max over doc tokens, split across two engines to beat the DVE-only
    1 elem/cycle/lane tensor_reduce bottleneck (both engines drain PSUM
    concurrently, block-interleaved within each group):
      block 0 of each group: DVE reduce_max directly from PSUM;
      blocks 1-3: ScalarE copy-converts PSUM fp32 -> SBUF fp16, then DVE
        folds pairwise with tensor_tensor max in 2x_1p mode (2 elem/
        cycle/lane, overlapping even-offset windows 180->92->46->24->12)
        and one small strided reduce_max.
  - PE: per-group selector matmul sums each query's 32 token-maxes:
    sel[128,4].T @ maxvals[128, 32d] -> outps[4, 32] -> DMA as [32q, 32d].
"""

import numpy as np
from contextlib import ExitStack

import concourse.mybir as mybir
import concourse.tile as tile
from concourse import bacc
from concourse.bass_utils import run_bass_kernel_spmd

H = 128            # head dim (contraction)
NQ, M = 32, 32     # queries, query tokens
NDOC, NTOK = 256, 180
NCORES = 8
DSHARD = NDOC // NCORES      # 32 docs per core
GROUPS, QPG = 8, 4           # query groups of 4; 4*32 = 128 partitions
BLK_DOCS = 8                 # docs per PSUM block
NBLK = DSHARD // BLK_DOCS    # 4 blocks per group
BLK = BLK_DOCS * NTOK        # 1440 sim columns per block
GCOLS = DSHARD * NTOK        # 5760 doc-token columns per core
FOLD_DOCS = DSHARD - BLK_DOCS  # 24 docs per group take the fold route

# overlapping pairwise-fold splits: (len, off0, off1); all offsets even so
# fp16 packed pairs stay 4B-aligned for the DVE 2x_1p mode. Stop at 24
# columns: the final strided reduce_max costs less than two more
# overhead-dominated fold levels.
FOLD_PLAN = [(92, 0, 88), (46, 0, 46), (24, 0, 22)]

_CACHE = {}


# combined input tensor layout: [qt (1024) | dt (5760) | sel (4)] = 6788 cols
QT_OFF = 0
DT_OFF = GROUPS * 128          # 1024
SEL_OFF = DT_OFF + GCOLS       # 6784
IN_COLS = SEL_OFF + QPG        # 6788
# DMA plan: qt on the sync queue; doc blocks on the scalar queue in
# consumption order (block 0 first, so it lands in parallel with qt).
# Each slice <= 2560 cols so every descriptor stays one per partition
# row under the 5120B cap. No gpsimd-issued DMA: first SWDGE use pays a
# ~6us Q7 code load and a dge_drain at exit.
DMA_PLAN = [("sync", 0, DT_OFF),
            ("scalar", DT_OFF, DT_OFF + BLK),
            ("scalar", DT_OFF + BLK, DT_OFF + 3 * BLK),
            ("scalar", DT_OFF + 3 * BLK, IN_COLS)]


def _ldw_key(inst):
    w = inst.ins[0]
    return (str(w.memref), w.offset, str(w.ap), str(w.dtype))


def _dedupe_ldweights(nc):
    """Drop LDWEIGHTS whose stationary operand is already loaded.

    Tile lowers every matmul to an Ldweights+Matmult pair; our inner loops
    issue 12 matmuls per Q-group against the same stationary operand, and
    the redundant reloads serialize the PE (~480ns/matmul vs ~215ns).
    Walrus codegen emits exactly the BIR instructions, so removing a
    duplicate here removes the HW reload. Sems on a removed Ldweights are
    merged onto the following instruction (same engine, later point - the
    waits still guard the matmul, the updates still fire)."""
    for blk in nc.m.functions[0].blocks:
        insts = blk.instructions
        last_key = None
        keep = []
        pending_waits, pending_updates = [], []
        for inst in insts:
            if inst.opcode == "Ldweights":
                key = _ldw_key(inst)
                if key == last_key:
                    si = inst.sync_info
                    if si is not None:
                        pending_waits.extend(si.on_wait or [])
                        pending_updates.extend(si.on_update or [])
                    continue        # drop duplicate
                last_key = key
            elif inst.opcode == "Matmult":
                if pending_waits or pending_updates:
                    si = inst.sync_info
                    if si is None:
                        import concourse.mybir as _mybir
                        inst.sync_info = _mybir.SyncInfo(
                            on_wait=pending_waits, on_update=pending_updates)
                    else:
                        si.on_wait = list(si.on_wait or []) + pending_waits
                        si.on_update = list(si.on_update or []) + pending_updates
                    pending_waits, pending_updates = [], []
            elif getattr(inst, "engine", None) == mybir.EngineType.PE:
                last_key = None
            keep.append(inst)
        assert not pending_waits and not pending_updates
        if len(keep) != len(insts):
            blk.instructions[:] = keep


def _build():
    nc = bacc.Bacc(None, target_bir_lowering=False)
    f16 = mybir.dt.float16
    inp = nc.dram_tensor("inp", [H, IN_COLS], f16, kind="ExternalInput")
    scores = nc.dram_tensor("scores", [NQ, DSHARD], mybir.dt.float32,
                            kind="ExternalOutput")

    with ExitStack() as ctx:
        tc = ctx.enter_context(tile.TileContext(nc))
        singles = ctx.enter_context(tc.tile_pool(name="singles", bufs=1))
        psums = ctx.enter_context(tc.tile_pool(name="psums", bufs=2,
                                               space="PSUM"))
        gbufs = ctx.enter_context(tc.tile_pool(name="gbufs", bufs=3))
        folds = ctx.enter_context(tc.tile_pool(name="folds", bufs=2))
        outp = ctx.enter_context(tc.tile_pool(name="outp", bufs=1,
                                              space="PSUM"))

        in_sb = singles.tile([H, IN_COLS], f16)
        q_sb = in_sb[:, QT_OFF:QT_OFF + GROUPS * 128]
        d_sb = in_sb[:, DT_OFF:DT_OFF + GCOLS]
        sel_sb = in_sb[:, SEL_OFF:SEL_OFF + QPG]
        maxv = singles.tile([128, GROUPS, DSHARD], f16)

        for eng_name, lo, hi in DMA_PLAN:
            eng = {"sync": nc.sync, "scalar": nc.scalar,
                   "gpsimd": nc.gpsimd}[eng_name]
            eng.dma_start(out=in_sb[:, lo:hi], in_=inp[:, lo:hi])

        mm_splits = [(0, 512), (512, 512), (1024, BLK - 1024)]

        def sim_block(g, b):
            """Matmul one [128, 1440] sim block for group g into PSUM."""
            qg = q_sb[:, g * 128:(g + 1) * 128]
            ps = psums.tile([128, BLK], mybir.dt.float32, tag="ps",
                            name=f"ps_{g}_{b}")
            for (o, w) in mm_splits:
                nc.tensor.matmul(ps[:, o:o + w], qg,
                                 d_sb[:, b * BLK + o: b * BLK + o + w],
                                 start=True, stop=True)
            return ps

        # Per group: block 0 -> DVE reduce_max straight from PSUM (emitted
        # first so DVE starts as early as possible); blocks 1-3 -> ScalarE
        # copy-convert to fp16 SBUF, then a DVE fold tree. The fold tree of
        # group g-1 is emitted AFTER group g's direct reduce, so on the DVE
        # queue each group's PSUM-freeing reduce runs before the (slot-free)
        # fold work - both engines drain PSUM concurrently.
        outps = outp.tile([QPG, GROUPS * DSHARD], mybir.dt.float32)

        def emit_selector(g):
            # sum each query's 32 token-maxes for this group's docs; the
            # tiny 4-col selector reload costs the PE only a few ns.
            nc.tensor.matmul(outps[:, g * DSHARD:(g + 1) * DSHARD], sel_sb,
                             maxv[:, g, :], start=True, stop=True)

        def emit_folds(g, buf, docs, doc_lo):
            src = buf
            for li, (ln, o0, o1) in enumerate(FOLD_PLAN):
                dst = folds.tile([128, docs, ln], f16,
                                 tag=f"fold{li}", name=f"fold{li}_{g}_{doc_lo}")
                nc.vector.tensor_tensor(
                    dst, src[:, :, o0:o0 + ln], src[:, :, o1:o1 + ln],
                    op=mybir.AluOpType.max)
                src = dst
            nc.vector.reduce_max(maxv[:, g, doc_lo:doc_lo + docs], src,
                                 axis=mybir.AxisListType.X)
            if doc_lo + docs == DSHARD:
                emit_selector(g)

        # Block 0 of each group -> DVE direct reduce (emitted before the
        # previous group's folds, so the PSUM-freeing reduce runs first on
        # the DVE queue). The last group folds per-block to shorten the
        # serial fold tail.
        def emit_direct(g, b):
            ps = sim_block(g, b)
            nc.vector.reduce_max(
                maxv[:, g, 0:BLK_DOCS],
                ps.rearrange("p (d n) -> p d n", n=NTOK),
                axis=mybir.AxisListType.X)

        prev = None
        for g in range(GROUPS):
            last = g == GROUPS - 1
            buf = gbufs.tile([128, FOLD_DOCS, NTOK], f16, tag="buf",
                             name=f"buf_{g}")
            for b in range(NBLK):
                if b == 0:
                    if g == 0:
                        # hoist the first two groups' direct blocks so the
                        # DVE has queued work while the pipeline fills
                        emit_direct(0, 0)
                        emit_direct(1, 0)
                    elif g == 1:
                        pass       # already emitted
                    else:
                        emit_direct(g, b)
                    if prev is not None:
                        emit_folds(g - 1, prev, FOLD_DOCS, BLK_DOCS)
                        prev = None
                else:
                    ps = sim_block(g, b)
                    nc.scalar.copy(
                        out=buf[:, (b - 1) * BLK_DOCS:b * BLK_DOCS, :],
                        in_=ps.rearrange("p (d n) -> p d n", n=NTOK))
                    if last:
                        emit_folds(g, buf[:, (b - 1) * BLK_DOCS:b * BLK_DOCS, :],
                                   BLK_DOCS, b * BLK_DOCS)
            if not last:
                prev = buf


        scores_sb = singles.tile([QPG, GROUPS * DSHARD], mybir.dt.float32)
        nc.scalar.copy(out=scores_sb, in_=outps)
        nc.sync.dma_start(
            out=scores[:, :].rearrange("(g j) d -> j g d", j=QPG),
            in_=scores_sb.rearrange("j (g d) -> j g d", g=GROUPS))
    _dedupe_ldweights(nc)
    nc.finalize()
    return nc


def _get_program():
    if "nc" not in _CACHE:
        _CACHE["nc"] = _build()
    return _CACHE["nc"]


def _prep_inputs(Q, D, q_mask, d_mask):
    Qm = np.asarray(Q, np.float32) * np.asarray(q_mask, np.float32)[..., None]
    Dm = np.asarray(D, np.float32) * np.asarray(d_mask, np.float32)[..., None]

    qt = Qm.reshape(GROUPS, QPG, M, H).transpose(3, 0, 1, 2).reshape(
        H, GROUPS * 128).astype(np.float16)
    sel = np.repeat(np.eye(QPG, dtype=np.float32), M, axis=0).astype(np.float16)

    in_maps = []
    for c in range(NCORES):
        Dc = Dm[c * DSHARD:(c + 1) * DSHARD]          # [32, 180, 128]
        dtc = Dc.transpose(2, 0, 1).reshape(H, GCOLS).astype(np.float16)
        inp = np.empty((H, IN_COLS), dtype=np.float16)
        inp[:, QT_OFF:QT_OFF + GROUPS * 128] = qt
        inp[:, DT_OFF:DT_OFF + GCOLS] = dtc
        inp[:, SEL_OFF:SEL_OFF + QPG] = sel
        in_maps.append({"inp": inp})
    return in_maps


def run(Q, D, q_mask, d_mask, trace=False, **spmd_kwargs):
    """Run the sharded kernel; returns (scores [32,256] fp32, BassKernelResults)."""
    nc = _get_program()
    in_maps = _prep_inputs(Q, D, q_mask, d_mask)
    res = run_bass_kernel_spmd(nc, in_maps, core_ids=list(range(NCORES)),
                               trace=trace, **spmd_kwargs)
    full = np.empty((NQ, NDOC), dtype=np.float32)
    for c in range(NCORES):
        full[:, c * DSHARD:(c + 1) * DSHARD] = res.results[c]["scores"]
    return full, res


def kernel(Q, D, q_mask, d_mask):
    out, _ = run(Q, D, q_mask, d_mask, trace=False)
    return out

